# revision 6
# baseline (speedup 1.0000x reference)
"""Trainium2 Bass kernel for nn_BiTransition_41961830482675.

reference:
    graph0 -> graph0                      (identity pass-through)
    graph1 -> graph1 / rowsum(graph1)     (row-normalized adjacency)

Sharding: rows of graph1 split across 8 NeuronCores (1024 rows each).
Row-sum and division are fully row-local -> no communication.
graph0 is returned as-is on the host (the reference returns the input
object untouched), so no HBM traffic is spent on it.
"""

import numpy as np

import concourse.bass as bass
import concourse.bacc as bacc
import concourse.tile as tile
from concourse import mybir
from concourse.bass_utils import run_bass_kernel_spmd

N = 8192
N_CORES = 8
ROWS = N // N_CORES  # rows per core = 1024
P = 128              # SBUF partitions
N_BLOCKS = ROWS // P  # 8 row-blocks of [128, 8192] per core

_CACHED_NC = None


def _build_nc():
    # Bacc (not raw Bass): its compile() legalizes multi-wait instructions
    # into EventSemaphore ops, which the walrus codegen path requires.
    nc = bacc.Bacc("TRN2", target_bir_lowering=False, debug=False,
                   num_devices=N_CORES)
    g = nc.dram_tensor("g", [ROWS, N], mybir.dt.float32,
                       kind="ExternalInput").ap()
    o = nc.dram_tensor("o", [ROWS, N], mybir.dt.float32,
                       kind="ExternalOutput").ap()

    with tile.TileContext(nc) as tc:
        with tc.tile_pool(name="in", bufs=3) as in_pool, \
             tc.tile_pool(name="out", bufs=2) as out_pool, \
             tc.tile_pool(name="stat", bufs=N_BLOCKS) as stat_pool:
            for i in range(N_BLOCKS):
                t = in_pool.tile([P, N], mybir.dt.float32)
                nc.sync.dma_start(t[:], g[bass.ts(i, P), :])
                s = stat_pool.tile([P, 1], mybir.dt.float32)
                nc.vector.reduce_sum(s[:], t[:], axis=mybir.AxisListType.X)
                r = stat_pool.tile([P, 1], mybir.dt.float32)
                nc.vector.reciprocal(r[:], s[:])
                # Row-wise scale on DVE into a separate output tile; the
                # whole chain stays on one engine and each instruction
                # depends on at most one foreign sem lane (the DMA ISA
                # encoding has very few sync-wait slots).
                u = out_pool.tile([P, N], mybir.dt.float32)
                nc.vector.tensor_scalar_mul(u[:], t[:], r[:])
                nc.sync.dma_start(o[bass.ts(i, P), :], u[:])
    nc.compile()
    return nc


def _get_nc():
    global _CACHED_NC
    if _CACHED_NC is None:
        _CACHED_NC = _build_nc()
    return _CACHED_NC


def kernel(graph0: np.ndarray, graph1: np.ndarray, _trace=False):
    graph1 = np.ascontiguousarray(np.asarray(graph1))
    nc = _get_nc()
    in_maps = [{"g": graph1[c * ROWS:(c + 1) * ROWS]} for c in range(N_CORES)]
    res = run_bass_kernel_spmd(nc, in_maps, list(range(N_CORES)),
                               trace=_trace)
    out1 = np.concatenate([res.results[c]["o"] for c in range(N_CORES)],
                          axis=0)
    if _trace:
        kernel.last_results = res
    return (np.asarray(graph0), out1)


# revision 14
# speedup vs baseline: 1.1981x; 1.1981x over previous
"""Trainium2 Bass kernel for nn_BiTransition_41961830482675.

reference:
    graph0 -> graph0                      (identity pass-through)
    graph1 -> graph1 / rowsum(graph1)     (row-normalized adjacency)

Sharding: rows of graph1 split across 8 NeuronCores (1024 rows each).
Row-sum and division are fully row-local -> no communication.
graph0 is returned as-is on the host (the reference returns the input
object untouched), so no HBM traffic is spent on it.
"""

import numpy as np

import concourse.bass as bass
import concourse.bacc as bacc
import concourse.tile as tile
from concourse import mybir
from concourse.bass_utils import run_bass_kernel_spmd

N = 8192
N_CORES = 8
ROWS = N // N_CORES  # rows per core = 1024
P = 128              # SBUF partitions
N_BLOCKS = ROWS // P  # 8 row-blocks of [128, 8192] per core

_CACHED = {}


def _build_nc(ch=2048, in_bufs=None, out_bufs=None, store_eng="sync",
              last_ch=None):
    """Build the per-core program.

    ch: column-chunk width for load/reduce/scale/store tiling.
    store_eng: which HWDGE engine issues the store DMAs.
    last_ch: finer chunk width for the last row-block (shorter epilogue).
    """
    # Bacc (not raw Bass): its compile() legalizes multi-wait instructions
    # into EventSemaphore ops, which the walrus codegen path requires.
    nc = bacc.Bacc("TRN2", target_bir_lowering=False, debug=False,
                   num_devices=N_CORES)
    g = nc.dram_tensor("g", [ROWS, N], mybir.dt.float32,
                       kind="ExternalInput").ap()
    o = nc.dram_tensor("o", [ROWS, N], mybir.dt.float32,
                       kind="ExternalOutput").ap()

    f32 = mybir.dt.float32
    nch = N // ch
    if in_bufs is None:
        in_bufs = 3 * nch
    if out_bufs is None:
        out_bufs = 2 * nch
    if last_ch is None:
        last_ch = ch

    with tile.TileContext(nc) as tc:
        with tc.tile_pool(name="in", bufs=in_bufs) as in_pool, \
             tc.tile_pool(name="out", bufs=out_bufs) as out_pool, \
             tc.tile_pool(name="stat", bufs=4) as stat_pool:
            for i in range(N_BLOCKS):
                cw = last_ch if i == N_BLOCKS - 1 else ch
                ncw = N // cw
                store = getattr(nc, store_eng)
                # Chunked loads; each chunk's partial row-sum starts as
                # soon as that chunk lands, overlapping later loads.
                ts = []
                part = stat_pool.tile([P, ncw], f32, tag="part")
                for c in range(ncw):
                    t = in_pool.tile([P, cw], f32, tag="t")
                    nc.sync.dma_start(t[:], g[bass.ts(i, P), bass.ts(c, cw)])
                    ts.append(t)
                for c in range(ncw):
                    nc.vector.reduce_sum(part[:, c:c + 1], ts[c][:],
                                         axis=mybir.AxisListType.X)
                s = stat_pool.tile([P, 1], f32, tag="s")
                nc.vector.reduce_sum(s[:], part[:],
                                     axis=mybir.AxisListType.X)
                r = stat_pool.tile([P, 1], f32, tag="r")
                nc.vector.reciprocal(r[:], s[:])
                # Chunked scale (DVE tensor_scalar runs in 2x mode) and
                # store, so the store stream starts one chunk after the
                # row sums are known.
                for c in range(ncw):
                    u = out_pool.tile([P, cw], f32, tag="u")
                    nc.vector.tensor_scalar_mul(u[:], ts[c][:], r[:])
                    store.dma_start(o[bass.ts(i, P), bass.ts(c, cw)], u[:])
    nc.compile()
    return nc


def _build_raw(ch=2048, in_slots=3, out_slots=2):
    """Raw bacc pipeline with manual semaphores — no TileContext, so no
    start/end EVSEM butterflies or tail drain (~12-17us saved).

    Engines: SP issues loads, ACT issues stores (separate HWDGE rings),
    DVE does reduce/reciprocal/scale. Chunked as in the Tile version.
    Sem protocol (16 incs per DMA, 1 per DVE scale):
      ld after load  (i,c) == 16*(nch*i+c+1)
      dv after scale (i,c) ==     nch*i+c+1
      st after store (i,c) == 16*(nch*i+c+1)
    """
    nc = bacc.Bacc("TRN2", target_bir_lowering=False, debug=False,
                   num_devices=N_CORES)
    g = nc.dram_tensor("g", [ROWS, N], mybir.dt.float32,
                       kind="ExternalInput").ap()
    o = nc.dram_tensor("o", [ROWS, N], mybir.dt.float32,
                       kind="ExternalOutput").ap()
    f32 = mybir.dt.float32
    nch = N // ch
    X = mybir.AxisListType.X

    tb = [nc.alloc_sbuf_tensor(f"t{k}", [P, N], f32).ap()
          for k in range(in_slots)]
    ub = [nc.alloc_sbuf_tensor(f"u{k}", [P, N], f32).ap()
          for k in range(out_slots)]
    part = nc.alloc_sbuf_tensor("part", [P, nch], f32).ap()
    s = nc.alloc_sbuf_tensor("s", [P, 1], f32).ap()
    r = nc.alloc_sbuf_tensor("r", [P, 1], f32).ap()

    # Per-(slot, chunk) DMA-completion sems: successive DMAs sharing a sem
    # are already serialized by the pipeline's data deps, so cumulative
    # counts certify completion (one shared sem would interleave the +16s
    # of concurrent DMAs and certify nothing).
    ld = [[nc.alloc_semaphore(f"ld{k}_{c}") for c in range(nch)]
          for k in range(in_slots)]
    st = [[nc.alloc_semaphore(f"st{k}_{c}") for c in range(nch)]
          for k in range(out_slots)]
    dv = nc.alloc_semaphore("dv")  # DVE scale progress: nch*i+c+1
    q = nc.alloc_semaphore("q")    # DVE self-ordering chain

    with nc.Block() as block:

        @block.sync
        def _(sp):
            for i in range(N_BLOCKS):
                slot = i % in_slots
                for c in range(nch):
                    if i >= in_slots:
                        # chunk slot reuse: scale (i-in_slots, c) read it
                        sp.wait_ge(dv, nch * (i - in_slots) + c + 1)
                    sp.dma_start(
                        out=tb[slot][:, ch * c:ch * (c + 1)],
                        in_=g[bass.ts(i, P), bass.ts(c, ch)],
                    ).then_inc(ld[slot][c], 16)

        @block.vector
        def _(dve):
            # q: DVE self-ordering chain (6 ticks per block). Hardware is
            # already safe (in-order engine + per-op DRAIN); these waits
            # are always satisfied on arrival and only inform the race
            # detector's cross-op visibility model.
            for i in range(N_BLOCKS):
                slot = i % in_slots
                uslot = i % out_slots
                for c in range(nch):
                    dve.wait_ge(ld[slot][c], 16 * (i // in_slots + 1))
                    if i > 0:
                        dve.wait_ge(q, 6 * i)  # part WAR vs prev final sum
                    dve.reduce_sum(part[:, c:c + 1],
                                   tb[slot][:, ch * c:ch * (c + 1)],
                                   axis=X).then_inc(q, 1)
                dve.wait_ge(q, 6 * i + 4)
                dve.reduce_sum(s[:], part[:], axis=X).then_inc(q, 1)
                dve.wait_ge(q, 6 * i + 5)
                if i > 0:
                    dve.wait_ge(dv, nch * i)  # r WAR vs prev block scales
                dve.reciprocal(r[:], s[:]).then_inc(q, 1)
                if i >= out_slots:
                    # u slot reuse: stores of block i-out_slots done
                    j = i - out_slots
                    for c in range(nch):
                        dve.wait_ge(st[uslot][c], 16 * (j // out_slots + 1))
                for c in range(nch):
                    dve.wait_ge(q, 6 * i + 6)
                    dve.tensor_scalar_mul(
                        ub[uslot][:, ch * c:ch * (c + 1)],
                        tb[slot][:, ch * c:ch * (c + 1)], r[:],
                    ).then_inc(dv, 1)

        @block.scalar
        def _(act):
            for i in range(N_BLOCKS):
                uslot = i % out_slots
                for c in range(nch):
                    act.wait_ge(dv, nch * i + c + 1)
                    act.dma_start(
                        out=o[bass.ts(i, P), bass.ts(c, ch)],
                        in_=ub[uslot][:, ch * c:ch * (c + 1)],
                    ).then_inc(st[uslot][c], 16)
            # final drain: all stores of the last out_slots blocks
            for j in range(N_BLOCKS - out_slots, N_BLOCKS):
                for c in range(nch):
                    act.wait_ge(st[j % out_slots][c],
                                16 * (j // out_slots + 1))

    nc.compile()
    return nc


def _get_nc(**kw):
    key = tuple(sorted(kw.items()))
    if key not in _CACHED:
        builder = _build_raw if kw.pop("raw", False) else _build_nc
        _CACHED[key] = builder(**kw)
    return _CACHED[key]


def kernel(graph0: np.ndarray, graph1: np.ndarray, _trace=False, **kw):
    graph1 = np.ascontiguousarray(np.asarray(graph1))
    if not kw:
        kw = dict(raw=True, ch=2048)
    nc = _get_nc(**kw)
    in_maps = [{"g": graph1[c * ROWS:(c + 1) * ROWS]} for c in range(N_CORES)]
    res = run_bass_kernel_spmd(nc, in_maps, list(range(N_CORES)),
                               trace=_trace)
    out1 = np.concatenate([res.results[c]["o"] for c in range(N_CORES)],
                          axis=0)
    if _trace:
        kernel.last_results = res
    return (np.asarray(graph0), out1)
